# revision 35
# baseline (speedup 1.0000x reference)
"""Multi-head attention Bass kernel for Trainium2, 8 NeuronCores.

Problem: B=2, R=16, C=512, E=1024, H=16 heads, D=64.
  q,k,v = x @ w{q,k,v} + b{q,k,v}  (per-head attention)  out = ctx @ wo + bo

Sharding: pure data parallel over the B*R = 32 independent (batch,row)
sequences -> 4 sequences of 512 tokens per core. No collectives.

v2 (bf16 rewrite of the f32r baseline; tolerance is 2e-2, baseline hit
2.6e-4, bf16 lands ~2e-3):
  - all matmul operands bf16 (host casts x + weights to bf16): halves
    weight/x DMA, enables FWL fast weight loads, and lifts the f32r
    tiling restrictions so the row-tiled S head-pairs can overlap
  - x^T via PE transpose (bf16), Q^T/K^T produced transposed
  - V natural [tok, feat] with a ones column per head (PV emits the
    softmax denominator l for free in psum row 64)
  - S^T per head-pair as K=64 row-tile pairs; 2 kj chunks share a
    2-bank psum tile so each ACT exp covers [128,1024], P^T in bf16
  - softmax denominators batched per sequence: all 16 l rows are DMA'd
    into one [16,512] tile, ONE Ln + ONE Exp(-1) ACT op per sequence
    (ACT op cost is free-size bound, so 16 partitions cost the same as
    1), then per-head gpsimd partition_broadcast + DVE multiply
  - ctx^T assembled in SBUF (no DRAM bounce): even head written to
    partitions 0-63 directly, odd head staged then shifted to
    partitions 64-127 with a 64KB SBUF->SBUF DMA
  - output projection of sequence s-1 emitted after attention of
    sequence s: O-proj matmuls fill the PE while the next sequence's
    denominator chain and DMAs complete; no cold phase B, no tail
"""

import numpy as np
import ml_dtypes

import concourse.bacc as bacc
import concourse.mybir as mybir
import concourse.tile as tile
from concourse import bass_utils
from concourse.masks import make_identity

F32 = mybir.dt.float32
BF16 = mybir.dt.bfloat16

# Ln and Exp alternate per sequence (the batched denominator) amid the
# big softmax Exps. Left alone, the table-placement pass picks
# "exp_and_others" for Exp and "natural_log" for Ln, reloading the ACT
# tables (~2.7us) on every alternation. Restrict both functions to the
# one set that contains them together.
_orig_get_tables = bacc.get_activation_tables


def _combined_exp_ln_tables(arch):
    tabs = _orig_get_tables(arch)
    keep = "natural_log_exp_and_others"
    for name, fns in tabs.items():
        if name != keep:
            fns.discard(mybir.ActivationFunctionType.Exp)
            fns.discard(mybir.ActivationFunctionType.Ln)
    return tabs


bacc.get_activation_tables = _combined_exp_ln_tables

B, R, C, E, H = 2, 16, 512, 1024, 16
D = E // H            # 64
NCORES = 8
SEQS = (B * R) // NCORES   # 4 sequences per core
TCH = C // 128             # 4 token chunks per sequence
KCH = E // 128             # 8 contraction chunks
NCH = E // 512             # 2 output column chunks
PAIRS = H // 2             # 8 head pairs
SCALE = 1.0 / np.sqrt(D)   # folded into exp


def build_nc():
    nc = bacc.Bacc("TRN2", debug=False, num_devices=NCORES)

    xs_d = nc.dram_tensor("xs", [SEQS * C, E], BF16, kind="ExternalInput").ap()
    w_d = {}
    for w in ("wq", "wk", "wv", "wo"):
        w_d[w] = nc.dram_tensor(w, [E, E], BF16, kind="ExternalInput").ap()
    b_d = {}
    for b in ("bq", "bk", "bv", "bo"):
        b_d[b] = nc.dram_tensor(b, [E], F32, kind="ExternalInput").ap()
    os_d = nc.dram_tensor("os", [SEQS * C, E], F32, kind="ExternalOutput").ap()

    with tile.TileContext(nc) as tc:
        with (
            tc.tile_pool(name="consts", bufs=1) as cpool,
            tc.tile_pool(name="wpool", bufs=4) as wpool,
            tc.tile_pool(name="xin", bufs=8) as xinp,
            tc.tile_pool(name="xT", bufs=8) as xTp,
            tc.tile_pool(name="vsb", bufs=4) as vp,
            tc.tile_pool(name="qk", bufs=4) as qkp,
            tc.tile_pool(name="pt", bufs=7) as ptp,
            tc.tile_pool(name="cr", bufs=11) as crp,
            tc.tile_pool(name="lr", bufs=6) as lrp,
            tc.tile_pool(name="nrm", bufs=4) as nrmp,
            tc.tile_pool(name="stg", bufs=3) as stgp,
            tc.tile_pool(name="ctx", bufs=2) as ctxp,
            tc.tile_pool(name="osb", bufs=3) as osbp,
            tc.tile_pool(name="ps_pj", bufs=2, space="PSUM") as ps_pj,
            tc.tile_pool(name="ps_s", bufs=2, space="PSUM") as ps_s,
            tc.tile_pool(name="ps_c", bufs=2, space="PSUM") as ps_c,
        ):
            # -------- tiny bias DMAs FIRST (the PE's first instructions,
            # the bias-broadcast matmuls, depend on them; if they queue
            # behind the 8.4MB of weights the whole PE stream stalls) ----
            # per-partition bias layouts: t[p, j] = b[j*128 + p]
            bqt = cpool.tile([128, KCH], F32, name="bqt")
            bkt = cpool.tile([128, KCH], F32, name="bkt")
            for name, t in (("bq", bqt), ("bk", bkt)):
                src = b_d[name].rearrange("(j p) -> p j", p=128)
                nc.sync.dma_start(t[:], src)
            brow = cpool.tile([33, E], F32, name="brow")
            nc.sync.dma_start(brow[0:1, :], b_d["bv"].rearrange("(o e) -> o e", o=1))
            nc.sync.dma_start(brow[32:33, :], b_d["bo"].rearrange("(o e) -> o e", o=1))

            def load_x(s):
                # seq 0 is startup-critical: split its tile DMAs in column
                # halves so the first transposes start ~5us earlier
                nsplit = 2 if s == 0 else 1
                xin = []
                for t in range(TCH):
                    xt = xinp.tile([128, E], BF16, name=f"xin{s}_{t}", tag="xin")
                    w = E // nsplit
                    for q in range(nsplit):
                        nc.sync.dma_start(
                            xt[:, q * w:(q + 1) * w],
                            xs_d[s * C + t * 128: s * C + (t + 1) * 128,
                                 q * w:(q + 1) * w])
                    xin.append(xt)
                return xin

            def alloc_w(name):
                return wpool.tile([128, KCH * E], BF16, name=name, tag="w")

            def dma_w(t, name):
                for k in range(KCH):
                    nc.sync.dma_start(
                        t[:, k * E:(k + 1) * E], w_d[name][k * 128:(k + 1) * 128, :])
                return t

            # x(seq0) first (transposes touch PE earliest), then weights in
            # first-use order: wv (V proj), wq/wk, wo last (only needed
            # ~100us in)
            xin_next = load_x(0)
            wv_sb = dma_w(alloc_w("wv"), "wv")
            wq_sb = dma_w(alloc_w("wq"), "wq")
            wk_sb = dma_w(alloc_w("wk"), "wk")
            wo_sb = dma_w(alloc_w("wo"), "wo")

            # ---------------- constants ----------------
            ident = cpool.tile([128, 128], BF16, name="ident")
            make_identity(nc, ident[:])
            onesb = cpool.tile([128, 128], BF16, name="onesb")
            nc.vector.memset(onesb[:], 1.0)

            # bv/bo broadcast to all 128 partitions (free-dim biases) via
            # all-ones outer product matmul
            bvb = cpool.tile([128, E], F32, name="bvb")
            bob = cpool.tile([128, E], F32, name="bob")
            browr = cpool.tile([33, E], BF16, name="browr")
            nc.vector.tensor_copy(browr[0:1, :], brow[0:1, :])
            nc.vector.tensor_copy(browr[32:33, :], brow[32:33, :])
            for j, dst in ((0, bvb), (32, bob)):
                for n in range(NCH):
                    pb = ps_pj.tile([128, 512], F32, name=f"pb{j}{n}", tag="pj")
                    nc.tensor.matmul(
                        pb[:], onesb[j:j + 1, :],
                        browr[j:j + 1, n * 512:(n + 1) * 512],
                        start=True, stop=True)
                    nc.vector.tensor_copy(dst[:, n * 512:(n + 1) * 512], pb[:])

            # x^T via PE transpose: transpose 128x128 blocks through the
            # identity, copy psum -> bf16 sbuf tiles
            def transpose_x(s, xin):
                tiles = []
                for f in range(KCH):
                    ptr = ps_pj.tile([128, 512], BF16, name=f"ptr{s}_{f}", tag="pj")
                    for t in range(TCH):
                        nc.tensor.transpose(
                            ptr[:, t * 128:(t + 1) * 128],
                            xin[t][:, f * 128:(f + 1) * 128], ident[:])
                    xf = xTp.tile([128, 512], BF16, name=f"xT{s}_{f}", tag="xT")
                    nc.vector.tensor_copy(xf[:], ptr[:])
                    tiles.append(xf)
                return tiles

            ctx_sb = [None, None]   # double-buffered ctx^T [128, KCH*512]

            def attention(s, xT):
                # V projection: natural layout [tok 128, 16*(64+1)] with a
                # ones column appended per head (fused softmax-denominator)
                vsb = []
                for t in range(TCH):
                    vt = vp.tile([128, H * (D + 1)], BF16, name=f"v{s}_{t}", tag="v")
                    vt3 = vt[:].rearrange("p (h dd) -> p h dd", dd=D + 1)
                    nc.vector.tensor_copy(
                        vt3[:, :, D:D + 1],
                        onesb[:].rearrange("p (a b) -> p a b", b=1)[:, 0:H, :])
                    for n in range(NCH):
                        pv = ps_pj.tile([128, 512], F32, name=f"pv{s}_{t}{n}", tag="pj")
                        for k in range(KCH):
                            nc.tensor.matmul(
                                pv[:],
                                xT[k][:, t * 128:(t + 1) * 128],
                                wv_sb[:, k * E + n * 512: k * E + (n + 1) * 512],
                                start=(k == 0), stop=(k == KCH - 1))
                        hpc = E // NCH // D  # heads per chunk (8)
                        nc.vector.tensor_tensor(
                            vt3[:, n * hpc:(n + 1) * hpc, 0:D],
                            pv[:].rearrange("p (h d) -> p h d", d=D),
                            bvb[:].rearrange("p (h d) -> p h d", d=D)[:, n * hpc:(n + 1) * hpc, :],
                            op=mybir.AluOpType.add)
                    vsb.append(vt)

                # denominator batches: (head range, emitted after pair; None
                # = deferred, returned as a closure the caller emits after
                # the previous sequence's O-projection so the ~6us Ln/Exp/
                # pbc chain hides behind independent PE work instead of
                # head-of-line-blocking the DVE FIFO at the seq boundary).
                # One Ln+Exp ACT pair per batch (ACT op cost is free-size
                # bound, partitions are free). The last sequence has no
                # following work to hide behind, so it uses finer inline
                # batches to shorten the serial tail chain.
                if s < SEQS - 1:
                    batches = [(0, 8, 3), (8, 16, None)]
                else:
                    batches = [(0, 8, 3), (8, 12, 5), (12, 16, 7)]
                lr = {}
                crs = [None] * H

                ctx = ctxp.tile([128, KCH * 512], BF16, name=f"ctx{s}", tag="ctx")
                ctx_sb[s % 2] = ctx

                def normalize_batch(h0, h1):
                    lrg = lr[h0]
                    # ln in fp32 (absolute error in the exponent becomes
                    # relative error on 1/l), result back to bf16
                    lg = lrp.tile([h1 - h0, 512], F32, name=f"lg{s}_{h0}",
                                  tag="lr")
                    lrec = lrp.tile([h1 - h0, 512], BF16, name=f"lrec{s}_{h0}",
                                    tag="lr")
                    nc.scalar.activation(lg[:], lrg[:],
                                         mybir.ActivationFunctionType.Ln)
                    nc.scalar.activation(lrec[:], lg[:],
                                         mybir.ActivationFunctionType.Exp,
                                         scale=-1.0)
                    # only pbc lives on gpsimd (mixing op types on gpsimd
                    # forces ucode LIBRARY_RELOADs that wreck its queue);
                    # multiplies stay on DVE
                    for h in range(h0, h1):
                        f = h // 2
                        nt = nrmp.tile([64, 1024], BF16, name=f"nt{s}{h}", tag="nt")
                        nc.sync.dma_start(nt[0:1, 512:1024],
                                          lrec[h - h0:h - h0 + 1, :])
                        nc.gpsimd.partition_broadcast(nt[0:64, 0:512],
                                                      nt[0:1, 512:1024])
                        if h % 2 == 0:
                            nc.vector.tensor_tensor(
                                ctx[0:64, f * 512:(f + 1) * 512],
                                crs[h][0:64, :], nt[0:64, 0:512],
                                op=mybir.AluOpType.mult)
                        else:
                            st = stgp.tile([64, 512], BF16,
                                           name=f"st{s}{h}", tag="st")
                            nc.vector.tensor_tensor(
                                st[:], crs[h][0:64, :], nt[0:64, 0:512],
                                op=mybir.AluOpType.mult)
                            nc.sync.dma_start(
                                ctx[64:128, f * 512:(f + 1) * 512], st[:])
                        crs[h] = None

                for p in range(PAIRS):
                    for (h0, h1, ap) in batches:
                        if 2 * p == h0:
                            lr[h0] = lrp.tile([h1 - h0, 512], BF16,
                                              name=f"lr{s}_{h0}", tag="lr")
                    # Q^T / K^T for this feature pair [128 feat, 512 tok]
                    qkt = {}
                    for nm, wsb, bt in (("q", wq_sb, bqt), ("k", wk_sb, bkt)):
                        pq = ps_pj.tile([128, 512], F32, name=f"pq{nm}{s}_{p}", tag="pj")
                        for k in range(KCH):
                            nc.tensor.matmul(
                                pq[:],
                                wsb[:, k * E + p * 128: k * E + (p + 1) * 128],
                                xT[k][:],
                                start=(k == 0), stop=(k == KCH - 1))
                        qt = qkp.tile([128, 512], BF16, name=f"{nm}T{s}_{p}", tag="qk")
                        # bias-adds split across engines: Q on ACT (Identity
                        # + per-partition bias AP, keeps the DVE FIFO clear
                        # for psum drains), K on DVE (keeps ACT under the
                        # exp load in pair phases)
                        if nm == "q":
                            nc.scalar.activation(
                                qt[:], pq[:],
                                mybir.ActivationFunctionType.Identity,
                                bias=bt[:, p:p + 1])
                        else:
                            nc.vector.tensor_scalar_add(qt[:], pq[:],
                                                        bt[:, p:p + 1])
                        qkt[nm] = qt
                    QT, KT = qkt["q"], qkt["k"]

                    # S^T chunks + exp -> P^T, per head (row-tiled pairs).
                    # Two kj-chunks share one 2-bank psum tile so each exp
                    # covers [128,1024].
                    PT2 = [[None, None] for _ in range(2)]
                    for cp in range(TCH // 2):
                        pse = ps_s.tile([128, 1024], F32, name=f"pse{s}{p}{cp}", tag="s")
                        pso = ps_s.tile([128, 1024], F32, name=f"pso{s}{p}{cp}", tag="s")
                        for ci in range(2):
                            c = 2 * cp + ci
                            nc.tensor.matmul(
                                pse[:, ci * 512:(ci + 1) * 512],
                                KT[0:64, c * 128:(c + 1) * 128], QT[0:64, :],
                                start=True, stop=True, tile_position=(0, 0))
                            nc.tensor.matmul(
                                pso[:, ci * 512:(ci + 1) * 512],
                                KT[64:128, c * 128:(c + 1) * 128], QT[64:128, :],
                                start=True, stop=True, tile_position=(64, 0))
                        for hh, ps_t in ((0, pse), (1, pso)):
                            pt_t = ptp.tile([128, 1024], BF16,
                                            name=f"pt{s}{p}{cp}{hh}", tag="pt")
                            nc.scalar.activation(
                                pt_t[:], ps_t[:],
                                mybir.ActivationFunctionType.Exp, scale=float(SCALE))
                            PT2[hh][cp] = pt_t

                    # fused ctx^T + softmax-denominator per head:
                    # psum rows 0..63 = ctx^T, row 64 = l (ones column of V)
                    for hh in range(2):
                        h = 2 * p + hh
                        pc = ps_c.tile([65, 512], F32, name=f"pc{s}{p}{hh}", tag="c")
                        for c in range(TCH):
                            nc.tensor.matmul(
                                pc[:],
                                vsb[c][:, h * (D + 1):(h + 1) * (D + 1)],
                                PT2[hh][c // 2][:, (c % 2) * 512:(c % 2 + 1) * 512],
                                start=(c == 0), stop=(c == TCH - 1))
                        # unnormalized ctx^T + l row out of psum early (bf16:
                        # makes the normalize multiply all-16-bit; l loses
                        # ~0.4% which is well inside the 2e-2 budget); l row
                        # to the batch tile (DMA cannot read psum)
                        cr = crp.tile([65, 512], BF16, name=f"cr{s}{p}{hh}", tag="cr")
                        nc.vector.tensor_copy(cr[:], pc[:])
                        h0 = max(b[0] for b in batches if b[0] <= h)
                        nc.sync.dma_start(lr[h0][h - h0:h - h0 + 1, :],
                                          cr[64:65, :])
                        crs[h] = cr

                    for (h0, h1, ap) in batches:
                        if p == ap:
                            normalize_batch(h0, h1)
                deferred = [(h0, h1) for (h0, h1, ap) in batches if ap is None]
                return lambda: [normalize_batch(h0, h1) for h0, h1 in deferred]

            def o_proj(s):
                ctx = ctx_sb[s % 2]
                for t in range(TCH):
                    for n in range(NCH):
                        po = ps_pj.tile([128, 512], F32, name=f"po{s}{t}{n}",
                                        tag="pj")
                        for k in range(KCH):
                            nc.tensor.matmul(
                                po[:],
                                ctx[:, k * 512 + t * 128: k * 512 + (t + 1) * 128],
                                wo_sb[:, k * E + n * 512: k * E + (n + 1) * 512],
                                start=(k == 0), stop=(k == KCH - 1))
                        ob = osbp.tile([128, 512], F32, name=f"ob{s}{t}{n}", tag="ob")
                        nc.vector.tensor_tensor(
                            ob[:], po[:], bob[:, n * 512:(n + 1) * 512],
                            op=mybir.AluOpType.add)
                        # halved output DMAs: a full [128,512] fp32 chunk is
                        # 256KB = ~11us on one queue; halving shortens the
                        # critical last transfer
                        for q in range(2):
                            nc.sync.dma_start(
                                os_d[s * C + t * 128: s * C + (t + 1) * 128,
                                     n * 512 + q * 256: n * 512 + (q + 1) * 256],
                                ob[:, q * 256:(q + 1) * 256])

            # ------------- main loop: per-seq pipelined -------------
            # emission order per boundary: attention(s) [normalize g0
            # inline] -> o_proj(s-1) -> transposes(s+1) -> deferred
            # normalize g1(s) -> attention(s+1): the g1 chain's DVE
            # multiplies queue AFTER the o_proj/transpose psum drains, so
            # the PE never waits on the chain's serial latency.
            xT = transpose_x(0, xin_next)
            for s in range(SEQS):
                if s + 1 < SEQS:
                    xin_next = load_x(s + 1)
                norm_g1 = attention(s, xT)
                if s > 0:
                    o_proj(s - 1)
                if s + 1 < SEQS:
                    xT = transpose_x(s + 1, xin_next)
                norm_g1()
            o_proj(SEQS - 1)

    nc.compile()
    return nc


_NC_CACHE = {}


def get_nc():
    if "nc" not in _NC_CACHE:
        _NC_CACHE["nc"] = build_nc()
    return _NC_CACHE["nc"]


def make_in_maps(x, wq, bq, wk, bk, wv, bv, wo, bo):
    bf = ml_dtypes.bfloat16
    x = np.asarray(x, dtype=np.float32).astype(bf)
    args = {}
    for n, v in (("wq", wq), ("wk", wk), ("wv", wv), ("wo", wo)):
        args[n] = np.asarray(v, dtype=np.float32).astype(bf)
    for n, v in (("bq", bq), ("bk", bk), ("bv", bv), ("bo", bo)):
        args[n] = np.asarray(v, dtype=np.float32)
    xf = x.reshape(B * R, C, E)
    in_maps = []
    for c in range(NCORES):
        m = dict(args)
        m["xs"] = np.ascontiguousarray(
            xf[c * SEQS:(c + 1) * SEQS].reshape(SEQS * C, E))
        in_maps.append(m)
    return in_maps


def kernel(x, wq, bq, wk, bk, wv, bv, wo, bo):
    in_maps = make_in_maps(x, wq, bq, wk, bk, wv, bv, wo, bo)
    nc = get_nc()
    res = bass_utils.run_bass_kernel_spmd(
        nc, in_maps, core_ids=list(range(NCORES)))
    out = np.concatenate(
        [res.results[c]["os"].reshape(SEQS, C, E) for c in range(NCORES)], axis=0)
    return out.reshape(B, R, C, E).astype(np.float32)


# revision 36
# speedup vs baseline: 1.0314x; 1.0314x over previous
"""Multi-head attention Bass kernel for Trainium2, 8 NeuronCores.

Problem: B=2, R=16, C=512, E=1024, H=16 heads, D=64.
  q,k,v = x @ w{q,k,v} + b{q,k,v}  (per-head attention)  out = ctx @ wo + bo

Sharding: pure data parallel over the B*R = 32 independent (batch,row)
sequences -> 4 sequences of 512 tokens per core. No collectives.

v2 (bf16 rewrite of the f32r baseline; tolerance is 2e-2, baseline hit
2.6e-4, bf16 lands ~2e-3):
  - all matmul operands bf16 (host casts x + weights to bf16): halves
    weight/x DMA, enables FWL fast weight loads, and lifts the f32r
    tiling restrictions so the row-tiled S head-pairs can overlap
  - x^T via PE transpose (bf16), Q^T/K^T produced transposed
  - V natural [tok, feat] with a ones column per head (PV emits the
    softmax denominator l for free in psum row 64)
  - S^T per head-pair as K=64 row-tile pairs; 2 kj chunks share a
    2-bank psum tile so each ACT exp covers [128,1024], P^T in bf16
  - softmax denominators batched per sequence: all 16 l rows are DMA'd
    into one [16,512] tile, ONE Ln + ONE Exp(-1) ACT op per sequence
    (ACT op cost is free-size bound, so 16 partitions cost the same as
    1), then per-head gpsimd partition_broadcast + DVE multiply
  - ctx^T assembled in SBUF (no DRAM bounce): even head written to
    partitions 0-63 directly, odd head staged then shifted to
    partitions 64-127 with a 64KB SBUF->SBUF DMA
  - output projection of sequence s-1 emitted after attention of
    sequence s: O-proj matmuls fill the PE while the next sequence's
    denominator chain and DMAs complete; no cold phase B, no tail
"""

import numpy as np
import ml_dtypes

import concourse.bacc as bacc
import concourse.mybir as mybir
import concourse.tile as tile
from concourse import bass_utils
from concourse.masks import make_identity

F32 = mybir.dt.float32
BF16 = mybir.dt.bfloat16

# Ln and Exp alternate per sequence (the batched denominator) amid the
# big softmax Exps. Left alone, the table-placement pass picks
# "exp_and_others" for Exp and "natural_log" for Ln, reloading the ACT
# tables (~2.7us) on every alternation. Restrict both functions to the
# one set that contains them together.
_orig_get_tables = bacc.get_activation_tables


def _combined_exp_ln_tables(arch):
    tabs = _orig_get_tables(arch)
    keep = "natural_log_exp_and_others"
    for name, fns in tabs.items():
        if name != keep:
            fns.discard(mybir.ActivationFunctionType.Exp)
            fns.discard(mybir.ActivationFunctionType.Ln)
    return tabs


bacc.get_activation_tables = _combined_exp_ln_tables

B, R, C, E, H = 2, 16, 512, 1024, 16
D = E // H            # 64
NCORES = 8
SEQS = (B * R) // NCORES   # 4 sequences per core
TCH = C // 128             # 4 token chunks per sequence
KCH = E // 128             # 8 contraction chunks
NCH = E // 512             # 2 output column chunks
PAIRS = H // 2             # 8 head pairs
SCALE = 1.0 / np.sqrt(D)   # folded into exp


def build_nc():
    nc = bacc.Bacc("TRN2", debug=False, num_devices=NCORES)

    xs_d = nc.dram_tensor("xs", [SEQS * C, E], BF16, kind="ExternalInput").ap()
    w_d = {}
    for w in ("wq", "wk", "wv", "wo"):
        w_d[w] = nc.dram_tensor(w, [E, E], BF16, kind="ExternalInput").ap()
    b_d = {}
    for b in ("bq", "bk", "bv", "bo"):
        b_d[b] = nc.dram_tensor(b, [E], F32, kind="ExternalInput").ap()
    os_d = nc.dram_tensor("os", [SEQS * C, E], F32, kind="ExternalOutput").ap()

    with tile.TileContext(nc) as tc:
        with (
            tc.tile_pool(name="consts", bufs=1) as cpool,
            tc.tile_pool(name="wpool", bufs=4) as wpool,
            tc.tile_pool(name="xin", bufs=8) as xinp,
            tc.tile_pool(name="xT", bufs=8) as xTp,
            tc.tile_pool(name="vsb", bufs=4) as vp,
            tc.tile_pool(name="qk", bufs=4) as qkp,
            tc.tile_pool(name="pt", bufs=7) as ptp,
            tc.tile_pool(name="cr", bufs=11) as crp,
            tc.tile_pool(name="lr", bufs=6) as lrp,
            tc.tile_pool(name="nrm", bufs=4) as nrmp,
            tc.tile_pool(name="stg", bufs=3) as stgp,
            tc.tile_pool(name="ctx", bufs=2) as ctxp,
            tc.tile_pool(name="osb", bufs=3) as osbp,
            tc.tile_pool(name="ps_pj", bufs=2, space="PSUM") as ps_pj,
            tc.tile_pool(name="ps_s", bufs=2, space="PSUM") as ps_s,
            tc.tile_pool(name="ps_c", bufs=2, space="PSUM") as ps_c,
        ):
            # -------- tiny bias DMAs FIRST (the PE's first instructions,
            # the bias-broadcast matmuls, depend on them; if they queue
            # behind the 8.4MB of weights the whole PE stream stalls) ----
            # per-partition bias layouts: t[p, j] = b[j*128 + p]
            bqt = cpool.tile([128, KCH], F32, name="bqt")
            bkt = cpool.tile([128, KCH], F32, name="bkt")
            for name, t in (("bq", bqt), ("bk", bkt)):
                src = b_d[name].rearrange("(j p) -> p j", p=128)
                nc.sync.dma_start(t[:], src)
            brow = cpool.tile([33, E], F32, name="brow")
            nc.sync.dma_start(brow[0:1, :], b_d["bv"].rearrange("(o e) -> o e", o=1))
            nc.sync.dma_start(brow[32:33, :], b_d["bo"].rearrange("(o e) -> o e", o=1))

            def load_x(s):
                # seq 0 is startup-critical: split its tile DMAs in column
                # halves so the first transposes start ~5us earlier
                nsplit = 2 if s == 0 else 1
                xin = []
                for t in range(TCH):
                    xt = xinp.tile([128, E], BF16, name=f"xin{s}_{t}", tag="xin")
                    w = E // nsplit
                    for q in range(nsplit):
                        nc.sync.dma_start(
                            xt[:, q * w:(q + 1) * w],
                            xs_d[s * C + t * 128: s * C + (t + 1) * 128,
                                 q * w:(q + 1) * w])
                    xin.append(xt)
                return xin

            def alloc_w(name):
                return wpool.tile([128, KCH * E], BF16, name=name, tag="w")

            def dma_w(t, name):
                for k in range(KCH):
                    nc.sync.dma_start(
                        t[:, k * E:(k + 1) * E], w_d[name][k * 128:(k + 1) * 128, :])
                return t

            # x(seq0) first (transposes touch PE earliest), then weights in
            # first-use order: wv (V proj), wq/wk, wo last (only needed
            # ~100us in)
            xin_next = load_x(0)
            wv_sb = dma_w(alloc_w("wv"), "wv")
            wq_sb = dma_w(alloc_w("wq"), "wq")
            wk_sb = dma_w(alloc_w("wk"), "wk")
            wo_sb = dma_w(alloc_w("wo"), "wo")

            # ---------------- constants ----------------
            ident = cpool.tile([128, 128], BF16, name="ident")
            make_identity(nc, ident[:])
            onesb = cpool.tile([128, 128], BF16, name="onesb")
            nc.vector.memset(onesb[:], 1.0)

            # bv/bo broadcast to all 128 partitions (free-dim biases) via
            # all-ones outer product matmul
            bvb = cpool.tile([128, E], F32, name="bvb")
            bob = cpool.tile([128, E], F32, name="bob")
            browr = cpool.tile([33, E], BF16, name="browr")
            nc.vector.tensor_copy(browr[0:1, :], brow[0:1, :])
            nc.vector.tensor_copy(browr[32:33, :], brow[32:33, :])
            for j, dst in ((0, bvb), (32, bob)):
                for n in range(NCH):
                    pb = ps_pj.tile([128, 512], F32, name=f"pb{j}{n}", tag="pj")
                    nc.tensor.matmul(
                        pb[:], onesb[j:j + 1, :],
                        browr[j:j + 1, n * 512:(n + 1) * 512],
                        start=True, stop=True)
                    nc.vector.tensor_copy(dst[:, n * 512:(n + 1) * 512], pb[:])

            # x^T via PE transpose: transpose 128x128 blocks through the
            # identity, copy psum -> bf16 sbuf tiles
            def transpose_x(s, xin):
                tiles = []
                for f in range(KCH):
                    ptr = ps_pj.tile([128, 512], BF16, name=f"ptr{s}_{f}", tag="pj")
                    for t in range(TCH):
                        nc.tensor.transpose(
                            ptr[:, t * 128:(t + 1) * 128],
                            xin[t][:, f * 128:(f + 1) * 128], ident[:])
                    xf = xTp.tile([128, 512], BF16, name=f"xT{s}_{f}", tag="xT")
                    nc.vector.tensor_copy(xf[:], ptr[:])
                    tiles.append(xf)
                return tiles

            ctx_sb = [None, None]   # double-buffered ctx^T [128, KCH*512]

            def attention(s, xT):
                # V projection: natural layout [tok 128, 16*(64+1)] with a
                # ones column appended per head (fused softmax-denominator)
                vsb = []
                for t in range(TCH):
                    vt = vp.tile([128, H * (D + 1)], BF16, name=f"v{s}_{t}", tag="v")
                    vt3 = vt[:].rearrange("p (h dd) -> p h dd", dd=D + 1)
                    nc.vector.tensor_copy(
                        vt3[:, :, D:D + 1],
                        onesb[:].rearrange("p (a b) -> p a b", b=1)[:, 0:H, :])
                    for n in range(NCH):
                        pv = ps_pj.tile([128, 512], F32, name=f"pv{s}_{t}{n}", tag="pj")
                        for k in range(KCH):
                            nc.tensor.matmul(
                                pv[:],
                                xT[k][:, t * 128:(t + 1) * 128],
                                wv_sb[:, k * E + n * 512: k * E + (n + 1) * 512],
                                start=(k == 0), stop=(k == KCH - 1))
                        hpc = E // NCH // D  # heads per chunk (8)
                        nc.vector.tensor_tensor(
                            vt3[:, n * hpc:(n + 1) * hpc, 0:D],
                            pv[:].rearrange("p (h d) -> p h d", d=D),
                            bvb[:].rearrange("p (h d) -> p h d", d=D)[:, n * hpc:(n + 1) * hpc, :],
                            op=mybir.AluOpType.add)
                    vsb.append(vt)

                # denominator batches: (head range, emitted after pair; None
                # = deferred, returned as a closure the caller emits after
                # the previous sequence's O-projection so the ~6us Ln/Exp/
                # pbc chain hides behind independent PE work instead of
                # head-of-line-blocking the DVE FIFO at the seq boundary).
                # One Ln+Exp ACT pair per batch (ACT op cost is free-size
                # bound, partitions are free). The last sequence has no
                # following work to hide behind, so it uses finer inline
                # batches to shorten the serial tail chain.
                if s < SEQS - 1:
                    batches = [(0, 8, 3), (8, 16, None)]
                else:
                    batches = [(0, 8, 3), (8, 12, 5), (12, 16, 7)]
                lr = {}
                crs = [None] * H

                ctx = ctxp.tile([128, KCH * 512], BF16, name=f"ctx{s}", tag="ctx")
                ctx_sb[s % 2] = ctx

                def normalize_batch(h0, h1):
                    lrg = lr[h0]
                    # ln in fp32 (absolute error in the exponent becomes
                    # relative error on 1/l), result back to bf16
                    lg = lrp.tile([h1 - h0, 512], F32, name=f"lg{s}_{h0}",
                                  tag="lr")
                    lrec = lrp.tile([h1 - h0, 512], BF16, name=f"lrec{s}_{h0}",
                                    tag="lr")
                    nc.scalar.activation(lg[:], lrg[:],
                                         mybir.ActivationFunctionType.Ln)
                    nc.scalar.activation(lrec[:], lg[:],
                                         mybir.ActivationFunctionType.Exp,
                                         scale=-1.0)
                    # only pbc lives on gpsimd (mixing op types on gpsimd
                    # forces ucode LIBRARY_RELOADs that wreck its queue);
                    # multiplies stay on DVE
                    for h in range(h0, h1):
                        f = h // 2
                        nt = nrmp.tile([64, 1024], BF16, name=f"nt{s}{h}", tag="nt")
                        nc.sync.dma_start(nt[0:1, 512:1024],
                                          lrec[h - h0:h - h0 + 1, :])
                        nc.gpsimd.partition_broadcast(nt[0:64, 0:512],
                                                      nt[0:1, 512:1024])
                        if h % 2 == 0:
                            nc.vector.tensor_tensor(
                                ctx[0:64, f * 512:(f + 1) * 512],
                                crs[h][0:64, :], nt[0:64, 0:512],
                                op=mybir.AluOpType.mult)
                        else:
                            st = stgp.tile([64, 512], BF16,
                                           name=f"st{s}{h}", tag="st")
                            nc.vector.tensor_tensor(
                                st[:], crs[h][0:64, :], nt[0:64, 0:512],
                                op=mybir.AluOpType.mult)
                            nc.sync.dma_start(
                                ctx[64:128, f * 512:(f + 1) * 512], st[:])
                        crs[h] = None

                for p in range(PAIRS):
                    for (h0, h1, ap) in batches:
                        if 2 * p == h0:
                            lr[h0] = lrp.tile([h1 - h0, 512], BF16,
                                              name=f"lr{s}_{h0}", tag="lr")
                    # Q^T / K^T for this feature pair [128 feat, 512 tok]
                    qkt = {}
                    for nm, wsb, bt in (("q", wq_sb, bqt), ("k", wk_sb, bkt)):
                        pq = ps_pj.tile([128, 512], F32, name=f"pq{nm}{s}_{p}", tag="pj")
                        for k in range(KCH):
                            nc.tensor.matmul(
                                pq[:],
                                wsb[:, k * E + p * 128: k * E + (p + 1) * 128],
                                xT[k][:],
                                start=(k == 0), stop=(k == KCH - 1))
                        qt = qkp.tile([128, 512], BF16, name=f"{nm}T{s}_{p}", tag="qk")
                        # bias-add on ACT (Identity + per-partition bias AP):
                        # the DVE FIFO is the scarce resource (psum drains
                        # head-of-line block behind anything slow), ACT has
                        # the headroom even at the exp load
                        nc.scalar.activation(
                            qt[:], pq[:],
                            mybir.ActivationFunctionType.Identity,
                            bias=bt[:, p:p + 1])
                        qkt[nm] = qt
                    QT, KT = qkt["q"], qkt["k"]

                    # S^T chunks + exp -> P^T, per head (row-tiled pairs).
                    # Two kj-chunks share one 2-bank psum tile so each exp
                    # covers [128,1024].
                    PT2 = [[None, None] for _ in range(2)]
                    for cp in range(TCH // 2):
                        pse = ps_s.tile([128, 1024], F32, name=f"pse{s}{p}{cp}", tag="s")
                        pso = ps_s.tile([128, 1024], F32, name=f"pso{s}{p}{cp}", tag="s")
                        for ci in range(2):
                            c = 2 * cp + ci
                            nc.tensor.matmul(
                                pse[:, ci * 512:(ci + 1) * 512],
                                KT[0:64, c * 128:(c + 1) * 128], QT[0:64, :],
                                start=True, stop=True, tile_position=(0, 0))
                            nc.tensor.matmul(
                                pso[:, ci * 512:(ci + 1) * 512],
                                KT[64:128, c * 128:(c + 1) * 128], QT[64:128, :],
                                start=True, stop=True, tile_position=(64, 0))
                        for hh, ps_t in ((0, pse), (1, pso)):
                            pt_t = ptp.tile([128, 1024], BF16,
                                            name=f"pt{s}{p}{cp}{hh}", tag="pt")
                            nc.scalar.activation(
                                pt_t[:], ps_t[:],
                                mybir.ActivationFunctionType.Exp, scale=float(SCALE))
                            PT2[hh][cp] = pt_t

                    # fused ctx^T + softmax-denominator per head:
                    # psum rows 0..63 = ctx^T, row 64 = l (ones column of V)
                    for hh in range(2):
                        h = 2 * p + hh
                        pc = ps_c.tile([65, 512], F32, name=f"pc{s}{p}{hh}", tag="c")
                        for c in range(TCH):
                            nc.tensor.matmul(
                                pc[:],
                                vsb[c][:, h * (D + 1):(h + 1) * (D + 1)],
                                PT2[hh][c // 2][:, (c % 2) * 512:(c % 2 + 1) * 512],
                                start=(c == 0), stop=(c == TCH - 1))
                        # unnormalized ctx^T + l row out of psum early (bf16:
                        # makes the normalize multiply all-16-bit; l loses
                        # ~0.4% which is well inside the 2e-2 budget); l row
                        # to the batch tile (DMA cannot read psum)
                        cr = crp.tile([65, 512], BF16, name=f"cr{s}{p}{hh}", tag="cr")
                        nc.vector.tensor_copy(cr[:], pc[:])
                        h0 = max(b[0] for b in batches if b[0] <= h)
                        nc.sync.dma_start(lr[h0][h - h0:h - h0 + 1, :],
                                          cr[64:65, :])
                        crs[h] = cr

                    for (h0, h1, ap) in batches:
                        if p == ap:
                            normalize_batch(h0, h1)
                deferred = [(h0, h1) for (h0, h1, ap) in batches if ap is None]
                return lambda: [normalize_batch(h0, h1) for h0, h1 in deferred]

            def o_proj(s):
                ctx = ctx_sb[s % 2]
                for t in range(TCH):
                    for n in range(NCH):
                        po = ps_pj.tile([128, 512], F32, name=f"po{s}{t}{n}",
                                        tag="pj")
                        for k in range(KCH):
                            nc.tensor.matmul(
                                po[:],
                                ctx[:, k * 512 + t * 128: k * 512 + (t + 1) * 128],
                                wo_sb[:, k * E + n * 512: k * E + (n + 1) * 512],
                                start=(k == 0), stop=(k == KCH - 1))
                        ob = osbp.tile([128, 512], F32, name=f"ob{s}{t}{n}", tag="ob")
                        nc.vector.tensor_tensor(
                            ob[:], po[:], bob[:, n * 512:(n + 1) * 512],
                            op=mybir.AluOpType.add)
                        # halved output DMAs: a full [128,512] fp32 chunk is
                        # 256KB = ~11us on one queue; halving shortens the
                        # critical last transfer
                        for q in range(2):
                            nc.sync.dma_start(
                                os_d[s * C + t * 128: s * C + (t + 1) * 128,
                                     n * 512 + q * 256: n * 512 + (q + 1) * 256],
                                ob[:, q * 256:(q + 1) * 256])

            # ------------- main loop: per-seq pipelined -------------
            # emission order per boundary: attention(s) [normalize g0
            # inline] -> o_proj(s-1) -> transposes(s+1) -> deferred
            # normalize g1(s) -> attention(s+1): the g1 chain's DVE
            # multiplies queue AFTER the o_proj/transpose psum drains, so
            # the PE never waits on the chain's serial latency.
            xT = transpose_x(0, xin_next)
            for s in range(SEQS):
                if s + 1 < SEQS:
                    xin_next = load_x(s + 1)
                norm_g1 = attention(s, xT)
                if s > 0:
                    o_proj(s - 1)
                if s + 1 < SEQS:
                    xT = transpose_x(s + 1, xin_next)
                norm_g1()
            o_proj(SEQS - 1)

    nc.compile()
    return nc


_NC_CACHE = {}


def get_nc():
    if "nc" not in _NC_CACHE:
        _NC_CACHE["nc"] = build_nc()
    return _NC_CACHE["nc"]


def make_in_maps(x, wq, bq, wk, bk, wv, bv, wo, bo):
    bf = ml_dtypes.bfloat16
    x = np.asarray(x, dtype=np.float32).astype(bf)
    args = {}
    for n, v in (("wq", wq), ("wk", wk), ("wv", wv), ("wo", wo)):
        args[n] = np.asarray(v, dtype=np.float32).astype(bf)
    for n, v in (("bq", bq), ("bk", bk), ("bv", bv), ("bo", bo)):
        args[n] = np.asarray(v, dtype=np.float32)
    xf = x.reshape(B * R, C, E)
    in_maps = []
    for c in range(NCORES):
        m = dict(args)
        m["xs"] = np.ascontiguousarray(
            xf[c * SEQS:(c + 1) * SEQS].reshape(SEQS * C, E))
        in_maps.append(m)
    return in_maps


def kernel(x, wq, bq, wk, bk, wv, bv, wo, bo):
    in_maps = make_in_maps(x, wq, bq, wk, bk, wv, bv, wo, bo)
    nc = get_nc()
    res = bass_utils.run_bass_kernel_spmd(
        nc, in_maps, core_ids=list(range(NCORES)))
    out = np.concatenate(
        [res.results[c]["os"].reshape(SEQS, C, E) for c in range(NCORES)], axis=0)
    return out.reshape(B, R, C, E).astype(np.float32)
